# revision 48
# baseline (speedup 1.0000x reference)
"""Bass/Trainium2 kernel for a 2-block single-head causal transformer.

Strategy (8 NeuronCores): data-parallel over batch (B=4 -> 4 core pairs),
sequence-parallel within each pair. Each core owns the interleaved global
query tiles {2j + t} (t = core parity), so the instruction stream is
identical on every core; all per-core variation (tokens, positional rows,
causal edge masks, vocab slice) is input data.

Per block, each core computes K^T / V for its own rows only; the halves are
exchanged with a pair-wise AllGather (rank order == parity order, so the
"stored" key order [even tiles | odd tiles] is core-invariant). Attention,
Wo, and the FFN then run on the core's own rows without further
communication. The final-token logits are computed with the vocab sharded
8 ways after a tiny 8-core AllGather of the last-row activations.

Everything is bf16 into the PE array with fp32 PSUM accumulation; softmax
runs in fp32 on PSUM-resident scores with deferred normalization.
"""

import sys

sys.path.insert(0, "/opt/trn_rl_repo")

import numpy as np
import ml_dtypes

import concourse.bass as bass
import concourse.mybir as mybir
import concourse.tile as tile
from concourse import bacc
from concourse.bass import IndirectOffsetOnAxis
from concourse.bass_utils import run_bass_kernel_spmd
from concourse.masks import make_identity

BF16 = mybir.dt.bfloat16
F32 = mybir.dt.float32
I32 = mybir.dt.int32
P = 128
NEG = -1.0e9


def _chunks(total, step):
    out = []
    off = 0
    while off < total:
        out.append((off, min(step, total - off)))
        off += step
    return out


def build_nc(S=2048, D=1024, H=4096, V=32000, n_cores=8, stage="full"):
    """Build the SPMD Bass program (identical on all cores).

    stage: "h0" | "kv" | "attn" | "block1" | "blocks" | "full" — truncate the
    program after the named phase and dump an intermediate to `dbg` (debug).
    """
    NJ = (S // P) // 2          # own q-tiles (slots) per core
    ND = D // P                 # d blocks
    NH = H // P                 # h blocks
    SO = S // 2                 # own rows per core
    KH0 = SO // 2               # h0 load half
    VS = V // n_cores           # vocab slice per core
    W1CH = min(8, NH)           # h-blocks per streamed w1 chunk
    QH = min(512, SO)           # q-half size for the FFN
    VC = 500 if VS % 500 == 0 else VS  # logits n-chunk
    pair_groups = [[2 * i, 2 * i + 1] for i in range(n_cores // 2)]
    all_group = [list(range(n_cores))]

    nc = bacc.Bacc("TRN2", target_bir_lowering=False, debug=False,
                   num_devices=n_cores)

    # ---- external inputs ----
    # h0T_own = (emb[tokens[own rows]] + pe[own rows]).T, staged on the host
    # pre-transposed into the [d mod 128, d block, own row] layout the kernel
    # uses, so no PE transposes are needed at startup
    h0T_own = nc.dram_tensor("h0T_own", [P, ND, SO], BF16, kind="ExternalInput")
    mask = nc.dram_tensor("mask", [P, NJ, 2 * P], BF16, kind="ExternalInput")
    # all weights are staged host-side in p-major layouts so every DMA is 128
    # descriptors of 8-16KB contiguous per partition (the naive rearranged
    # loads generate ~190k sub-2KB descriptors and saturate the DMA queues)
    CW = min(8, H // P) * P     # w1 chunk width
    NCH = H // CW
    NC_OUT = VS // VC
    wts = {}
    for l in (1, 2):
        wts[l, "wk"] = nc.dram_tensor(f"l{l}_wk", [P, ND, D], BF16, kind="ExternalInput")
        wts[l, "wv"] = nc.dram_tensor(f"l{l}_wv", [P, ND, D], BF16, kind="ExternalInput")
        wts[l, "wo"] = nc.dram_tensor(f"l{l}_wo", [P, ND, D], BF16, kind="ExternalInput")
        wts[l, "w1"] = nc.dram_tensor(f"l{l}_w1", [NCH, P, ND, CW], BF16, kind="ExternalInput")
        wts[l, "w2"] = nc.dram_tensor(f"l{l}_w2", [ND, P, NH, P], BF16, kind="ExternalInput")
    w_out = nc.dram_tensor("w_out", [NC_OUT, P, ND, VC], BF16, kind="ExternalInput")
    logits = nc.dram_tensor("logits", [4, VS], F32, kind="ExternalOutput")
    dbg = None
    if stage != "full":
        dbg = nc.dram_tensor("dbg", [P, ND, S], BF16, kind="ExternalOutput")

    with tile.TileContext(nc) as tc:
        with (
            tc.tile_pool(name="big", bufs=2) as big,          # kT / v / midT
            tc.tile_pool(name="own", bufs=1) as own_p,        # own hT
            tc.tile_pool(name="hat", bufs=1) as hat_p,        # h_attnT
            tc.tile_pool(name="res", bufs=1) as res_p,        # h_resT
            tc.tile_pool(name="w", bufs=2) as w_p,            # streamed weights
            tc.tile_pool(name="attn", bufs=4) as attn_p,
            tc.tile_pool(name="attnT", bufs=1) as attnT_p,
            tc.tile_pool(name="small", bufs=4) as small_p,    # evict staging
            tc.tile_pool(name="misc", bufs=2) as misc_p,
            tc.tile_pool(name="const", bufs=1) as const_p,
            tc.tile_pool(name="ps_mm", bufs=2, space="PSUM") as ps_mm,
            tc.tile_pool(name="ps_sc", bufs=1, space="PSUM") as ps_sc,
            tc.tile_pool(name="ps_av", bufs=1, space="PSUM") as ps_av_p,
            tc.tile_pool(name="dram", bufs=2, space="DRAM") as dram_p,
        ):
            mask_sb = const_p.tile([P, NJ, 2 * P], BF16, tag="mask")
            nc.sync.dma_start(mask_sb[:], mask[:])

            ident = const_p.tile([P, P], BF16, tag="ident")
            make_identity(nc, ident[:])

            def pe_transpose(dst_ap, src_ap):
                # PE transpose (128x128 bf16) + DVE copy back to SBUF
                pst = ps_mm.tile([P, P], BF16, tag="mm")
                nc.tensor.transpose(pst[:], src_ap, ident[:])
                nc.vector.tensor_copy(dst_ap, pst[:])

            def logits_prologue():
                # 8-core AllGather of the last token's activations; transpose
                # [128p, ND] -> [ND, 128] first so the DRAM write (and the
                # gathered reads) are contiguous in d-order. Only the AllGather
                # is issued here; the consumers are emitted after the FFN so
                # the collective latency hides under the remaining FFN work.
                lc_t = misc_p.tile([ND, P], BF16, tag="lct")
                ps_lc = ps_mm.tile([P, 512], BF16, tag="mm")
                nc.tensor.transpose(ps_lc[:ND, :P], last_col[:], ident[:])
                nc.vector.tensor_copy(lc_t[:], ps_lc[:ND, :P])
                cc_l_in = dram_p.tile([D], BF16, tag="ccl")
                cc_l_out = dram_p.tile([n_cores, D], BF16, tag="cclo")
                nc.sync.dma_start(cc_l_in[:].rearrange("(i p) -> i p", p=P), lc_t[:])
                nc.gpsimd.collective_compute(
                    "AllGather", mybir.AluOpType.bypass,
                    replica_groups=all_group,
                    ins=[cc_l_in[:].opt()], outs=[cc_l_out[:].opt()],
                )
                return cc_l_out

            def logits_lhsT(cc_l_out):
                # rows 1,3,5,7 hold batches 0..3 (odd cores own the last row)
                h_last = misc_p.tile([4, ND, P], BF16, tag="hlast", bufs=2)
                nc.sync.dma_start(
                    h_last[:],
                    cc_l_out[:].rearrange("r (i p) -> r i p", p=P)[1::2],
                )
                lhsT = const_p.tile([P, ND, 4], BF16, tag="lhsT")
                for i in range(ND):
                    ps_t = ps_mm.tile([P, 512], BF16, tag="mm")
                    nc.tensor.transpose(ps_t[:, :4], h_last[:, i, :], ident[:4, :4])
                    nc.vector.tensor_copy(lhsT[:, i, :], ps_t[:, :4])
                return lhsT

            # ---------------- h0 load (pre-transposed on host) ----------------
            # split by column halves: the first kT groups only need columns
            # 0:KH0, so they start as soon as the first half lands
            own_hT = own_p.tile([P, ND, SO], BF16, tag="own")
            for hh in range(2):
                nc.sync.dma_start(
                    own_hT[:, :, hh * KH0 : (hh + 1) * KH0],
                    h0T_own[:, :, hh * KH0 : (hh + 1) * KH0],
                )

            if stage == "h0":
                nc.sync.dma_start(dbg[:, :, :SO], own_hT[:])

            # ---------------- transformer blocks ----------------
            if stage == "h0":
                blocks = ()
            elif stage in ("kv", "attn", "block1"):
                blocks = (1,)
            else:
                blocks = (1, 2)
            for l in blocks:
                # weights ride the scalar HWDGE queue; the sync queue is kept
                # clear for the latency-critical collective staging DMAs
                wk_sb = w_p.tile([P, ND, D], BF16, tag="w")
                nc.scalar.dma_start(wk_sb[:], wts[l, "wk"][:])
                wv_sb = w_p.tile([P, ND, D], BF16, tag="w")
                nc.scalar.dma_start(wv_sb[:], wts[l, "wv"][:])

                # K^T / V for own rows, AllGathered in column/row halves,
                # interleaved k0, v0, k1, v1 so each AllGather fires as early
                # as possible and overlaps the next half's compute. The
                # staging buffers are p-major so each eviction/load is one
                # DMA of 128 contiguous 8KB descriptors.
                KH = SO // 2            # own-column half for the k AllGather
                MH = NJ // 2            # own s-tiles per v AllGather half
                cc_in_k = [dram_p.tile([P, ND, KH], BF16, tag=f"cck{h}", name=f"cck{h}") for h in range(2)]
                cc_out_k = [dram_p.tile([2, P, ND, KH], BF16, tag=f"ccko{h}", name=f"ccko{h}") for h in range(2)]
                cc_in_v = [dram_p.tile([P, MH, D], BF16, tag=f"ccv{h}", name=f"ccv{h}") for h in range(2)]
                cc_out_v = [dram_p.tile([2, P, MH, D], BF16, tag=f"ccvo{h}", name=f"ccvo{h}") for h in range(2)]

                for hh in range(2):
                    # kT_own columns [hh*KH : (hh+1)*KH]
                    stg_k = small_p.tile([P, ND, KH], BF16, tag="stg", bufs=2)
                    for off0, n in _chunks(KH, 512):
                        off = hh * KH + off0
                        for i in range(ND):
                            ps = ps_mm.tile([P, 512], F32, tag="mm")
                            for k in range(ND):
                                nc.tensor.matmul(
                                    ps[:, :n],
                                    wk_sb[:, k, i * P : (i + 1) * P],
                                    own_hT[:, k, off : off + n],
                                    start=(k == 0),
                                    stop=(k == ND - 1),
                                )
                            nc.vector.tensor_copy(
                                stg_k[:, i, off0 : off0 + n], ps[:, :n],
                            )
                    nc.scalar.dma_start(cc_in_k[hh][:], stg_k[:])
                    nc.gpsimd.collective_compute(
                        "AllGather", mybir.AluOpType.bypass,
                        replica_groups=pair_groups,
                        ins=[cc_in_k[hh][:].opt()], outs=[cc_out_k[hh][:].opt()],
                    )

                for hh in range(2):
                    # v_own s-tiles [hh*MH : (hh+1)*MH]
                    stg_v = small_p.tile([P, MH, D], BF16, tag="stg", bufs=2)
                    for m0 in range(MH):
                        m = hh * MH + m0
                        for off, n in _chunks(D, 512):
                            ps = ps_mm.tile([P, 512], F32, tag="mm")
                            for k in range(ND):
                                nc.tensor.matmul(
                                    ps[:, :n],
                                    own_hT[:, k, m * P : (m + 1) * P],
                                    wv_sb[:, k, off : off + n],
                                    start=(k == 0),
                                    stop=(k == ND - 1),
                                )
                            nc.vector.tensor_copy(
                                stg_v[:, m0, off : off + n], ps[:, :n],
                            )
                    nc.scalar.dma_start(cc_in_v[hh][:], stg_v[:])
                    nc.gpsimd.collective_compute(
                        "AllGather", mybir.AluOpType.bypass,
                        replica_groups=pair_groups,
                        ins=[cc_in_v[hh][:].opt()], outs=[cc_out_v[hh][:].opt()],
                    )

                # quarter loads ride the sync queue (evictions + exps live on
                # the scalar sequencer, so a load waiting on a late AllGather
                # can never park the softmax); order matches AllGather
                # completion order
                kT_q = [None] * 4
                v_q = [None] * 4
                for hh in range(2):
                    for q in (0 + hh, 2 + hh):
                        r = q // 2
                        t = big.tile([P, ND, KH], BF16, tag=f"kv{q}", name=f"kTq{q}")
                        nc.sync.dma_start(t[:], cc_out_k[hh][r])
                        kT_q[q] = t
                for hh in range(2):
                    for q in (0 + hh, 2 + hh):
                        r = q // 2
                        t = big.tile([P, MH, D], BF16, tag=f"kv{q}", name=f"vq{q}")
                        nc.sync.dma_start(t[:], cc_out_v[hh][r])
                        v_q[q] = t

                if stage == "kv":
                    for q in range(4):
                        nc.sync.dma_start(
                            dbg[:, :, q * KH : (q + 1) * KH], kT_q[q][:],
                        )
                    break

                # prefetch wo while attention runs (scalar: its WAR wait on
                # the wv slot resolves before the first exp is needed)
                wo_sb = w_p.tile([P, ND, D], BF16, tag="w")
                nc.scalar.dma_start(wo_sb[:], wts[l, "wo"][:])

                h_attnT = hat_p.tile([P, ND, SO], BF16, tag="hat")

                # ---------------- attention, software-pipelined over slots:
                # slot j's scores/softmax overlap slot j-1's attn@v on PE
                pend = {}

                def attn_head(j):
                    W1 = P * (j + 1)
                    ps_s = ps_sc.tile([P, S], F32, tag="sc")
                    # scores: two ranges (rank0 keys at [0:W1], rank1 at [SO:])
                    for base in (0, SO):
                        for off, n in _chunks(W1, min(512, KH)):
                            q = 2 * (base // SO) + (off // KH)
                            lo = off % KH
                            for k in range(ND):
                                nc.tensor.matmul(
                                    ps_s[:, base + off : base + off + n],
                                    own_hT[:, k, j * P : (j + 1) * P],
                                    kT_q[q][:, k, lo : lo + n],
                                    start=(k == 0),
                                    stop=(k == ND - 1),
                                )
                    # causal edge masks (one edge tile per range)
                    nc.vector.tensor_add(
                        ps_s[:, W1 - P : W1], ps_s[:, W1 - P : W1],
                        mask_sb[:, j, 0:P],
                    )
                    nc.vector.tensor_add(
                        ps_s[:, SO + W1 - P : SO + W1],
                        ps_s[:, SO + W1 - P : SO + W1],
                        mask_sb[:, j, P : 2 * P],
                    )
                    # softmax over both ranges; the attn tile is sized to
                    # 2*W1 columns (small slots get a small tag) so 8 tiles
                    # fit in SBUF at once. Big slots (W1>512) first evacuate
                    # the scores into the idle mm/av PSUM banks at DVE copy
                    # speed so the next head's matmuls reclaim the sc banks
                    # early; their max/exp then run off the PE critical path.
                    if W1 <= KH:
                        attn = attn_p.tile([P, 2, KH], BF16, tag="attnS", bufs=4)
                        sc2 = ps_s[:].rearrange("p (r s) -> p r s", s=SO)[:, :, :W1]
                        negmax = misc_p.tile([P, 1], F32, tag="negmax", bufs=8)
                        nc.vector.reduce_max(negmax[:], sc2,
                                             axis=mybir.AxisListType.XY,
                                             negate=True)
                        lsum = misc_p.tile([P, 1], F32, tag="lsum", bufs=8)
                        nc.scalar.activation(attn[:, :, :W1], sc2,
                                             mybir.ActivationFunctionType.Exp,
                                             bias=negmax[:], scale=1.0,
                                             accum_out=lsum[:])
                    else:
                        attn = attn_p.tile([P, 2, SO], BF16, tag="attnB", bufs=4)
                        cps = []
                        cp0a = ps_mm.tile([P, 512], F32, tag="mm")
                        nc.vector.tensor_copy(cp0a[:], ps_s[:, 0:512])
                        cps.append((cp0a[:], attn[:, 0, 0:512]))
                        cp0b = ps_mm.tile([P, 512], F32, tag="mm")
                        nc.vector.tensor_copy(cp0b[:, : W1 - 512], ps_s[:, 512:W1])
                        cps.append((cp0b[:, : W1 - 512], attn[:, 0, 512:W1]))
                        cp1 = ps_av_p.tile([P, 1024], F32, tag="av")
                        nc.vector.tensor_copy(cp1[:, :W1], ps_s[:, SO : SO + W1])
                        cps.append((cp1[:, :W1], attn[:, 1, :W1]))
                        negmax = misc_p.tile([P, 1], F32, tag="negmax", bufs=8)
                        for ci, (src, _) in enumerate(cps):
                            if ci == 0:
                                nc.vector.reduce_max(negmax[:], src,
                                                     axis=mybir.AxisListType.X,
                                                     negate=True)
                            else:
                                mp = misc_p.tile([P, 1], F32, tag="mpart", bufs=4)
                                nc.vector.reduce_max(mp[:], src,
                                                     axis=mybir.AxisListType.X,
                                                     negate=True)
                                nc.vector.tensor_tensor(negmax[:], negmax[:],
                                                        mp[:],
                                                        op=mybir.AluOpType.min)
                        lsum = misc_p.tile([P, 1], F32, tag="lsum", bufs=8)
                        for ci, (src, dst) in enumerate(cps):
                            if ci == 0:
                                nc.scalar.activation(
                                    dst, src, mybir.ActivationFunctionType.Exp,
                                    bias=negmax[:], scale=1.0, accum_out=lsum[:])
                            else:
                                lp = misc_p.tile([P, 1], F32, tag="lpart", bufs=4)
                                nc.scalar.activation(
                                    dst, src, mybir.ActivationFunctionType.Exp,
                                    bias=negmax[:], scale=1.0, accum_out=lp[:])
                                nc.vector.tensor_add(lsum[:], lsum[:], lp[:])
                    inv_l = misc_p.tile([P, 1], F32, tag="invl", bufs=8)
                    nc.vector.reciprocal(inv_l[:], lsum[:])
                    pend[j] = (attn, inv_l)

                def attn_tail(j):
                    attn, inv_l = pend.pop(j)
                    attnT = attnT_p.tile([P, 2 * NJ, P], BF16, tag="attnT")
                    for r in range(2):
                        for kk in range(j + 1):
                            pe_transpose(
                                attnT[:, r * NJ + kk, :],
                                attn[:, r, kk * P : (kk + 1) * P],
                            )
                    # attn @ v -> h_attn [q, D] (natural), deferred 1/l scale
                    ps_av = ps_av_p.tile([P, 1024], F32, tag="av")
                    for off, n in _chunks(D, 512):
                        first = True
                        for r in range(2):
                            for kk in range(j + 1):
                                g = r * NJ + kk
                                nc.tensor.matmul(
                                    ps_av[:, off : off + n],
                                    attnT[:, g, :],
                                    v_q[g // MH][:, g % MH, off : off + n],
                                    start=first,
                                    stop=(r == 1 and kk == j),
                                )
                                first = False
                    h_attn = misc_p.tile([P, D], BF16, tag="hattn")
                    nc.vector.tensor_scalar_mul(h_attn[:], ps_av[:, :D], inv_l[:])
                    # transpose into h_attnT columns for this slot
                    for i in range(ND):
                        pe_transpose(
                            h_attnT[:, i, j * P : (j + 1) * P],
                            h_attn[:, i * P : (i + 1) * P],
                        )

                # all heads, then all tails: k (both halves) is gathered
                # before any v half lands, so every head is runnable before
                # the first tail's v dependency — the PE FIFO never parks a
                # v-dependent tail in front of runnable heads
                for j in range(NJ):
                    attn_head(j)
                for j in range(NJ):
                    attn_tail(j)

                if stage == "attn":
                    nc.sync.dma_start(dbg[:, :, :SO], h_attnT[:])
                    break

                # ---------------- Wo + residual -> h_resT
                h_resT = res_p.tile([P, ND, SO], BF16, tag="res")
                for i in range(ND):
                    for off, n in _chunks(SO, 512):
                        ps = ps_mm.tile([P, 512], F32, tag="mm")
                        for k in range(ND):
                            nc.tensor.matmul(
                                ps[:, :n],
                                wo_sb[:, k, i * P : (i + 1) * P],
                                h_attnT[:, k, off : off + n],
                                start=(k == 0),
                                stop=(k == ND - 1),
                            )
                        nc.vector.tensor_add(
                            h_resT[:, i, off : off + n], ps[:, :n],
                            own_hT[:, i, off : off + n],
                        )

                # ---------------- FFN (per q-half; streamed w1/w2)
                # block 2 runs halves in reverse so the half holding the
                # final token finishes first -> the logits AllGather and
                # w_out streaming overlap the remaining FFN work
                own_hT_next = own_p.tile([P, ND, SO], BF16, tag="own")
                if l == 2 and stage == "full":
                    last_col = misc_p.tile([P, ND], BF16, tag="lastcol")
                n_w1ch = (NH + W1CH - 1) // W1CH
                qchunks = _chunks(SO, QH)
                if l == 2:
                    qchunks = qchunks[::-1]
                NHQ = NH // 4           # h-blocks per midT quarter
                for qoff, qn in qchunks:
                    midT = [big.tile([P, NHQ, QH], BF16, tag=f"kv{q}", name=f"midT{q}")
                            for q in range(4)]
                    for ch in range(n_w1ch):
                        hb0 = ch * W1CH
                        nhb = min(W1CH, NH - hb0)
                        w1_sb = w_p.tile([P, ND, W1CH * P], BF16, tag="w")
                        nc.scalar.dma_start(w1_sb[:, :, : nhb * P], wts[l, "w1"][ch])
                        for hb in range(nhb):
                            g = hb0 + hb
                            ps = ps_mm.tile([P, 512], F32, tag="mm")
                            for k in range(ND):
                                nc.tensor.matmul(
                                    ps[:, :qn],
                                    w1_sb[:, k, hb * P : (hb + 1) * P],
                                    h_resT[:, k, qoff : qoff + qn],
                                    start=(k == 0),
                                    stop=(k == ND - 1),
                                )
                            nc.vector.tensor_scalar_max(
                                midT[g // NHQ][:, g % NHQ, :qn], ps[:, :qn], 0.0,
                            )
                    for i in range(ND):
                        w2_sb = w_p.tile([P, NH, P], BF16, tag="w")
                        nc.sync.dma_start(w2_sb[:], wts[l, "w2"][i])
                        ps = ps_mm.tile([P, 512], F32, tag="mm")
                        for hb in range(NH):
                            nc.tensor.matmul(
                                ps[:, :qn],
                                w2_sb[:, hb, :],
                                midT[hb // NHQ][:, hb % NHQ, :qn],
                                start=(hb == 0),
                                stop=(hb == NH - 1),
                            )
                        nc.vector.tensor_add(
                            own_hT_next[:, i, qoff : qoff + qn], ps[:, :qn],
                            h_resT[:, i, qoff : qoff + qn],
                        )
                        if l == 2 and stage == "full" and qoff + qn == SO:
                            # last token's activations, kept in a tiny tile so
                            # the logits path doesn't wait on the whole FFN
                            nc.vector.tensor_add(
                                last_col[:, i : i + 1], ps[:, qn - 1 : qn],
                                h_resT[:, i, SO - 1 : SO],
                            )
                    if l == 2 and stage == "full" and qoff + qn == SO:
                        # emit the logits AllGather prologue here so the PE
                        # transposes interleave into the remaining FFN work
                        # and the 8-core collective overlaps it
                        lgp = logits_prologue()
                own_hT = own_hT_next
                if stage == "block1":
                    nc.sync.dma_start(dbg[:, :, :SO], own_hT[:])
                    break

            if stage == "blocks":
                nc.sync.dma_start(dbg[:, :, :SO], own_hT[:])

            if stage == "full":
                lhsT = logits_lhsT(lgp)
                # w_out streamed in VC-wide chunks through the kv pool slots;
                # the first chunks' slots free mid-way through block-2 FFN so
                # most of the stream overlaps compute
                wo_ts = []
                for ci, (off, n) in enumerate(_chunks(VS, VC)):
                    wo_t = big.tile([P, ND, VC], BF16, tag=f"kv{ci % 4}", name=f"wot{ci}")
                    eng = nc.sync if ci % 2 == 0 else nc.scalar
                    eng.dma_start(wo_t[:, :, :n], w_out[ci])
                    wo_ts.append(wo_t)
                # col-tiled: 4 vocab chunks run concurrently in 4 distinct
                # 32-wide column groups of the PE array (M=4 each), each
                # accumulating in its own bank of the (now idle) sc PSUM tile
                lchunks = _chunks(VS, VC)
                for g0 in range(0, len(lchunks), 4):
                    grp = lchunks[g0 : g0 + 4]
                    psg = ps_sc.tile([P, S], F32, tag="sc", name=f"lgps{g0}")
                    for k in range(ND):
                        for t, (off, n) in enumerate(grp):
                            nc.tensor.matmul(
                                psg[32 * t : 32 * t + 4, 512 * t : 512 * t + n],
                                lhsT[:, k, :], wo_ts[g0 + t][:, k, :n],
                                start=(k == 0), stop=(k == ND - 1),
                                tile_position=(0, 32 * t),
                            )
                    for t, (off, n) in enumerate(grp):
                        lg = misc_p.tile([4, VC], F32, tag="lg", bufs=2)
                        nc.vector.tensor_copy(
                            lg[:, :n],
                            psg[32 * t : 32 * t + 4, 512 * t : 512 * t + n],
                        )
                        nc.sync.dma_start(logits[:, off : off + n], lg[:, :n])

    nc.compile()
    return nc


# ----------------------------------------------------------------------------
# host side
# ----------------------------------------------------------------------------

def make_in_maps(tokens, emb, pe, weights, S=2048, D=1024, H=4096, V=32000,
                 n_cores=8):
    """weights: dict with l{1,2}_{wk,wv,wo,w1,w2} and w_out (fp32 numpy)."""
    bf = ml_dtypes.bfloat16
    NJ = (S // P) // 2
    VS = V // n_cores
    ND, NH = D // P, H // P
    CW = min(8, NH) * P
    NCH = H // CW
    VC = 500 if VS % 500 == 0 else VS
    NC_OUT = VS // VC
    emb_f = np.ascontiguousarray(emb, dtype=np.float32)
    pe_f = np.asarray(pe, dtype=np.float32)
    scale = 1.0 / np.sqrt(float(D))

    def pmaj(w):
        # [K, N] -> [P, K//P, N]: per-partition-contiguous weight layout
        K, N = w.shape
        return np.ascontiguousarray(w.reshape(K // P, P, N).transpose(1, 0, 2))

    w_bf = {}
    for l in (1, 2):
        wk = np.asarray(weights[f"l{l}_wk"], np.float32) * scale
        w_bf[f"l{l}_wk"] = pmaj(wk).astype(bf)
        w_bf[f"l{l}_wv"] = pmaj(np.asarray(weights[f"l{l}_wv"], np.float32)).astype(bf)
        w_bf[f"l{l}_wo"] = pmaj(np.asarray(weights[f"l{l}_wo"], np.float32)).astype(bf)
        w1 = np.asarray(weights[f"l{l}_w1"], np.float32)      # [D, H]
        w_bf[f"l{l}_w1"] = np.ascontiguousarray(
            w1.reshape(ND, P, NCH, CW).transpose(2, 1, 0, 3)
        ).astype(bf)                                          # [NCH, P, ND, CW]
        w2 = np.asarray(weights[f"l{l}_w2"], np.float32)      # [H, D]
        w_bf[f"l{l}_w2"] = np.ascontiguousarray(
            w2.reshape(NH, P, ND, P).transpose(2, 1, 0, 3)
        ).astype(bf)                                          # [ND, P, NH, P]
    w_out_f = np.asarray(weights["w_out"], np.float32)        # [D, V]

    tokens = np.asarray(tokens)
    B = tokens.shape[0]
    in_maps = []
    tri = np.triu(np.full((P, P), NEG, np.float32), k=1)  # [q, k] mask
    for c in range(n_cores):
        b, t = c // 2, c % 2
        own_rows = np.concatenate(
            [np.arange((2 * j + t) * P, (2 * j + t + 1) * P) for j in range(NJ)]
        )
        tok_own = tokens[b, own_rows].astype(np.int64)
        h0 = emb_f[tok_own] + pe_f[own_rows]                  # [SO, D]
        ND = D // P
        h0T_own = np.ascontiguousarray(
            h0.T.reshape(ND, P, len(own_rows)).transpose(1, 0, 2)
        ).astype(bf)                                          # [P, ND, SO]
        mask = np.zeros((NJ, P, 2 * P), np.float32)
        for j in range(NJ):
            if t == 0:
                mask[j, :, :P] = tri
                mask[j, :, P:] = NEG
            else:
                mask[j, :, P:] = tri
        w_out_c = w_out_f[:, c * VS : (c + 1) * VS]           # [D, VS]
        w_out_c = np.ascontiguousarray(
            w_out_c.reshape(ND, P, NC_OUT, VC).transpose(2, 1, 0, 3)
        ).astype(bf)                                          # [NC_OUT, P, ND, VC]
        in_map = {
            "h0T_own": h0T_own,
            "mask": np.ascontiguousarray(mask.transpose(1, 0, 2)).astype(bf),
            "w_out": w_out_c,
        }
        in_map.update(w_bf)
        in_maps.append(in_map)
    return in_maps


_NC_CACHE = {}


def _get_nc(key=(2048, 1024, 4096, 32000, 8)):
    if key not in _NC_CACHE:
        _NC_CACHE[key] = build_nc(*key)
    return _NC_CACHE[key]


def kernel(tokens, emb, pe, l1_wk, l1_wv, l1_wo, l1_w1, l1_w2,
           l2_wk, l2_wv, l2_wo, l2_w1, l2_w2, w_out):
    S = int(np.asarray(tokens).shape[1])
    D = int(np.asarray(emb).shape[1])
    H = int(np.asarray(l1_w1).shape[1])
    V = int(np.asarray(emb).shape[0])
    n_cores = 8
    nc = _get_nc((S, D, H, V, n_cores))
    weights = dict(
        l1_wk=l1_wk, l1_wv=l1_wv, l1_wo=l1_wo, l1_w1=l1_w1, l1_w2=l1_w2,
        l2_wk=l2_wk, l2_wv=l2_wv, l2_wo=l2_wo, l2_w1=l2_w1, l2_w2=l2_w2,
        w_out=w_out,
    )
    in_maps = make_in_maps(tokens, emb, pe, weights, S, D, H, V, n_cores)
    try:
        res = run_bass_kernel_spmd(nc, in_maps, core_ids=list(range(n_cores)))
    except Exception:
        # a previous crashed run can leave the device wedged; one retry
        # (fresh NRT session) clears it
        import os
        os.environ.setdefault("NEURON_RT_RESET_CORES", "1")
        res = run_bass_kernel_spmd(nc, in_maps, core_ids=list(range(n_cores)))
    VS = V // n_cores
    out = np.zeros((np.asarray(tokens).shape[0], V), np.float32)
    for c in range(n_cores):
        out[:, c * VS : (c + 1) * VS] = res.results[c]["logits"]
    return out



# revision 49
# speedup vs baseline: 1.0268x; 1.0268x over previous
"""Bass/Trainium2 kernel for a 2-block single-head causal transformer.

Strategy (8 NeuronCores): data-parallel over batch (B=4 -> 4 core pairs),
sequence-parallel within each pair. Each core owns the interleaved global
query tiles {2j + t} (t = core parity), so the instruction stream is
identical on every core; all per-core variation (tokens, positional rows,
causal edge masks, vocab slice) is input data.

Per block, each core computes K^T / V for its own rows only; the halves are
exchanged with a pair-wise AllGather (rank order == parity order, so the
"stored" key order [even tiles | odd tiles] is core-invariant). Attention,
Wo, and the FFN then run on the core's own rows without further
communication. The final-token logits are computed with the vocab sharded
8 ways after a tiny 8-core AllGather of the last-row activations.

Everything is bf16 into the PE array with fp32 PSUM accumulation; softmax
runs in fp32 on PSUM-resident scores with deferred normalization.
"""

import sys

sys.path.insert(0, "/opt/trn_rl_repo")

import numpy as np
import ml_dtypes

import concourse.bass as bass
import concourse.mybir as mybir
import concourse.tile as tile
from concourse import bacc
from concourse.bass import IndirectOffsetOnAxis
from concourse.bass_utils import run_bass_kernel_spmd
from concourse.masks import make_identity

BF16 = mybir.dt.bfloat16
F32 = mybir.dt.float32
I32 = mybir.dt.int32
P = 128
NEG = -1.0e9


def _chunks(total, step):
    out = []
    off = 0
    while off < total:
        out.append((off, min(step, total - off)))
        off += step
    return out


def build_nc(S=2048, D=1024, H=4096, V=32000, n_cores=8, stage="full"):
    """Build the SPMD Bass program (identical on all cores).

    stage: "h0" | "kv" | "attn" | "block1" | "blocks" | "full" — truncate the
    program after the named phase and dump an intermediate to `dbg` (debug).
    """
    NJ = (S // P) // 2          # own q-tiles (slots) per core
    ND = D // P                 # d blocks
    NH = H // P                 # h blocks
    SO = S // 2                 # own rows per core
    KH0 = SO // 2               # h0 load half
    VS = V // n_cores           # vocab slice per core
    W1CH = min(8, NH)           # h-blocks per streamed w1 chunk
    QH = min(512, SO)           # q-half size for the FFN
    VC = 500 if VS % 500 == 0 else VS  # logits n-chunk
    pair_groups = [[2 * i, 2 * i + 1] for i in range(n_cores // 2)]
    all_group = [list(range(n_cores))]

    nc = bacc.Bacc("TRN2", target_bir_lowering=False, debug=False,
                   num_devices=n_cores)

    # ---- external inputs ----
    # h0T_own = (emb[tokens[own rows]] + pe[own rows]).T, staged on the host
    # pre-transposed into the [d mod 128, d block, own row] layout the kernel
    # uses, so no PE transposes are needed at startup
    h0T_own = nc.dram_tensor("h0T_own", [P, ND, SO], BF16, kind="ExternalInput")
    mask = nc.dram_tensor("mask", [P, NJ, 2 * P], BF16, kind="ExternalInput")
    # all weights are staged host-side in p-major layouts so every DMA is 128
    # descriptors of 8-16KB contiguous per partition (the naive rearranged
    # loads generate ~190k sub-2KB descriptors and saturate the DMA queues)
    CW = min(8, H // P) * P     # w1 chunk width
    NCH = H // CW
    NC_OUT = VS // VC
    wts = {}
    for l in (1, 2):
        wts[l, "wk"] = nc.dram_tensor(f"l{l}_wk", [P, ND, D], BF16, kind="ExternalInput")
        wts[l, "wv"] = nc.dram_tensor(f"l{l}_wv", [P, ND, D], BF16, kind="ExternalInput")
        wts[l, "wo"] = nc.dram_tensor(f"l{l}_wo", [P, ND, D], BF16, kind="ExternalInput")
        wts[l, "w1"] = nc.dram_tensor(f"l{l}_w1", [NCH, P, ND, CW], BF16, kind="ExternalInput")
        wts[l, "w2"] = nc.dram_tensor(f"l{l}_w2", [ND, P, NH, P], BF16, kind="ExternalInput")
    w_out = nc.dram_tensor("w_out", [NC_OUT, P, ND, VC], BF16, kind="ExternalInput")
    logits = nc.dram_tensor("logits", [4, VS], F32, kind="ExternalOutput")
    dbg = None
    if stage != "full":
        dbg = nc.dram_tensor("dbg", [P, ND, S], BF16, kind="ExternalOutput")

    with tile.TileContext(nc) as tc:
        with (
            tc.tile_pool(name="big", bufs=2) as big,          # kT / v / midT
            tc.tile_pool(name="own", bufs=1) as own_p,        # own hT
            tc.tile_pool(name="hat", bufs=1) as hat_p,        # h_attnT
            tc.tile_pool(name="res", bufs=1) as res_p,        # h_resT
            tc.tile_pool(name="w", bufs=2) as w_p,            # streamed weights
            tc.tile_pool(name="attn", bufs=4) as attn_p,
            tc.tile_pool(name="attnT", bufs=1) as attnT_p,
            tc.tile_pool(name="small", bufs=4) as small_p,    # evict staging
            tc.tile_pool(name="misc", bufs=2) as misc_p,
            tc.tile_pool(name="const", bufs=1) as const_p,
            tc.tile_pool(name="ps_mm", bufs=2, space="PSUM") as ps_mm,
            tc.tile_pool(name="ps_sc", bufs=1, space="PSUM") as ps_sc,
            tc.tile_pool(name="ps_av", bufs=1, space="PSUM") as ps_av_p,
            tc.tile_pool(name="dram", bufs=2, space="DRAM") as dram_p,
        ):
            mask_sb = const_p.tile([P, NJ, 2 * P], BF16, tag="mask")
            nc.sync.dma_start(mask_sb[:], mask[:])

            ident = const_p.tile([P, P], BF16, tag="ident")
            make_identity(nc, ident[:])

            def pe_transpose(dst_ap, src_ap):
                # PE transpose (128x128 bf16) + DVE copy back to SBUF
                pst = ps_mm.tile([P, P], BF16, tag="mm")
                nc.tensor.transpose(pst[:], src_ap, ident[:])
                nc.vector.tensor_copy(dst_ap, pst[:])

            def logits_prologue():
                # 8-core AllGather of the last token's activations; transpose
                # [128p, ND] -> [ND, 128] first so the DRAM write (and the
                # gathered reads) are contiguous in d-order. Only the AllGather
                # is issued here; the consumers are emitted after the FFN so
                # the collective latency hides under the remaining FFN work.
                lc_t = misc_p.tile([ND, P], BF16, tag="lct")
                ps_lc = ps_mm.tile([P, 512], BF16, tag="mm")
                nc.tensor.transpose(ps_lc[:ND, :P], last_col[:], ident[:])
                nc.vector.tensor_copy(lc_t[:], ps_lc[:ND, :P])
                cc_l_in = dram_p.tile([D], BF16, tag="ccl")
                cc_l_out = dram_p.tile([n_cores, D], BF16, tag="cclo")
                nc.sync.dma_start(cc_l_in[:].rearrange("(i p) -> i p", p=P), lc_t[:])
                nc.gpsimd.collective_compute(
                    "AllGather", mybir.AluOpType.bypass,
                    replica_groups=all_group,
                    ins=[cc_l_in[:].opt()], outs=[cc_l_out[:].opt()],
                )
                return cc_l_out

            def logits_lhsT(cc_l_out):
                # rows 1,3,5,7 hold batches 0..3 (odd cores own the last row)
                h_last = misc_p.tile([4, ND, P], BF16, tag="hlast", bufs=2)
                nc.sync.dma_start(
                    h_last[:],
                    cc_l_out[:].rearrange("r (i p) -> r i p", p=P)[1::2],
                )
                lhsT = const_p.tile([P, ND, 4], BF16, tag="lhsT")
                for i in range(ND):
                    ps_t = ps_mm.tile([P, 512], BF16, tag="mm")
                    nc.tensor.transpose(ps_t[:, :4], h_last[:, i, :], ident[:4, :4])
                    nc.vector.tensor_copy(lhsT[:, i, :], ps_t[:, :4])
                return lhsT

            # ---------------- h0 load (pre-transposed on host) ----------------
            # split by column halves: the first kT groups only need columns
            # 0:KH0, so they start as soon as the first half lands
            own_hT = own_p.tile([P, ND, SO], BF16, tag="own")
            for hh in range(2):
                nc.sync.dma_start(
                    own_hT[:, :, hh * KH0 : (hh + 1) * KH0],
                    h0T_own[:, :, hh * KH0 : (hh + 1) * KH0],
                )

            if stage == "h0":
                nc.sync.dma_start(dbg[:, :, :SO], own_hT[:])

            # ---------------- transformer blocks ----------------
            if stage == "h0":
                blocks = ()
            elif stage in ("kv", "attn", "block1"):
                blocks = (1,)
            else:
                blocks = (1, 2)
            for l in blocks:
                # weights ride the scalar HWDGE queue; the sync queue is kept
                # clear for the latency-critical collective staging DMAs
                wk_sb = w_p.tile([P, ND, D], BF16, tag="w")
                nc.scalar.dma_start(wk_sb[:], wts[l, "wk"][:])
                wv_sb = w_p.tile([P, ND, D], BF16, tag="w")
                nc.scalar.dma_start(wv_sb[:], wts[l, "wv"][:])

                # K^T / V for own rows, AllGathered in column/row halves,
                # interleaved k0, v0, k1, v1 so each AllGather fires as early
                # as possible and overlaps the next half's compute. The
                # staging buffers are p-major so each eviction/load is one
                # DMA of 128 contiguous 8KB descriptors.
                KH = SO // 2            # own-column half for the k AllGather
                MH = NJ // 2            # own s-tiles per v AllGather half
                cc_in_k = [dram_p.tile([P, ND, KH], BF16, tag=f"cck{h}", name=f"cck{h}") for h in range(2)]
                cc_out_k = [dram_p.tile([2, P, ND, KH], BF16, tag=f"ccko{h}", name=f"ccko{h}") for h in range(2)]
                cc_in_v = [dram_p.tile([P, MH, D], BF16, tag=f"ccv{h}", name=f"ccv{h}") for h in range(2)]
                cc_out_v = [dram_p.tile([2, P, MH, D], BF16, tag=f"ccvo{h}", name=f"ccvo{h}") for h in range(2)]

                for hh in range(2):
                    # kT_own columns [hh*KH : (hh+1)*KH]
                    stg_k = small_p.tile([P, ND, KH], BF16, tag="stg", bufs=2)
                    for off0, n in _chunks(KH, 512):
                        off = hh * KH + off0
                        for i in range(ND):
                            ps = ps_mm.tile([P, 512], F32, tag="mm")
                            for k in range(ND):
                                nc.tensor.matmul(
                                    ps[:, :n],
                                    wk_sb[:, k, i * P : (i + 1) * P],
                                    own_hT[:, k, off : off + n],
                                    start=(k == 0),
                                    stop=(k == ND - 1),
                                )
                            nc.vector.tensor_copy(
                                stg_k[:, i, off0 : off0 + n], ps[:, :n],
                            )
                    nc.scalar.dma_start(cc_in_k[hh][:], stg_k[:])
                    nc.gpsimd.collective_compute(
                        "AllGather", mybir.AluOpType.bypass,
                        replica_groups=pair_groups,
                        ins=[cc_in_k[hh][:].opt()], outs=[cc_out_k[hh][:].opt()],
                    )

                for hh in range(2):
                    # v_own s-tiles [hh*MH : (hh+1)*MH]
                    stg_v = small_p.tile([P, MH, D], BF16, tag="stg", bufs=2)
                    for m0 in range(MH):
                        m = hh * MH + m0
                        for off, n in _chunks(D, 512):
                            ps = ps_mm.tile([P, 512], F32, tag="mm")
                            for k in range(ND):
                                nc.tensor.matmul(
                                    ps[:, :n],
                                    own_hT[:, k, m * P : (m + 1) * P],
                                    wv_sb[:, k, off : off + n],
                                    start=(k == 0),
                                    stop=(k == ND - 1),
                                )
                            nc.vector.tensor_copy(
                                stg_v[:, m0, off : off + n], ps[:, :n],
                            )
                    nc.scalar.dma_start(cc_in_v[hh][:], stg_v[:])
                    nc.gpsimd.collective_compute(
                        "AllGather", mybir.AluOpType.bypass,
                        replica_groups=pair_groups,
                        ins=[cc_in_v[hh][:].opt()], outs=[cc_out_v[hh][:].opt()],
                    )

                # quarter loads ride the sync queue (evictions + exps live on
                # the scalar sequencer, so a load waiting on a late AllGather
                # can never park the softmax); order matches AllGather
                # completion order
                kT_q = [None] * 4
                v_q = [None] * 4
                for hh in range(2):
                    for q in (0 + hh, 2 + hh):
                        r = q // 2
                        t = big.tile([P, ND, KH], BF16, tag=f"kv{q}", name=f"kTq{q}")
                        nc.sync.dma_start(t[:], cc_out_k[hh][r])
                        kT_q[q] = t
                for hh in range(2):
                    for q in (0 + hh, 2 + hh):
                        r = q // 2
                        t = big.tile([P, MH, D], BF16, tag=f"kv{q}", name=f"vq{q}")
                        nc.sync.dma_start(t[:], cc_out_v[hh][r])
                        v_q[q] = t

                if stage == "kv":
                    for q in range(4):
                        nc.sync.dma_start(
                            dbg[:, :, q * KH : (q + 1) * KH], kT_q[q][:],
                        )
                    break

                # prefetch wo while attention runs (scalar: its WAR wait on
                # the wv slot resolves before the first exp is needed)
                wo_sb = w_p.tile([P, ND, D], BF16, tag="w")
                nc.scalar.dma_start(wo_sb[:], wts[l, "wo"][:])

                h_attnT = hat_p.tile([P, ND, SO], BF16, tag="hat")

                # ---------------- attention, software-pipelined over slots:
                # slot j's scores/softmax overlap slot j-1's attn@v on PE
                pend = {}

                def attn_head(j):
                    W1 = P * (j + 1)
                    ps_s = ps_sc.tile([P, S], F32, tag="sc")
                    # scores: two ranges (rank0 keys at [0:W1], rank1 at
                    # [SO:]). Each range's causal mask + max reduce is issued
                    # right after its matmuls, so range 0's DVE work overlaps
                    # range 1's matmuls and the exp starts sooner.
                    mparts = []
                    for ri, base in enumerate((0, SO)):
                        for off, n in _chunks(W1, min(512, KH)):
                            q = 2 * ri + (off // KH)
                            lo = off % KH
                            for k in range(ND):
                                nc.tensor.matmul(
                                    ps_s[:, base + off : base + off + n],
                                    own_hT[:, k, j * P : (j + 1) * P],
                                    kT_q[q][:, k, lo : lo + n],
                                    start=(k == 0),
                                    stop=(k == ND - 1),
                                )
                        nc.vector.tensor_add(
                            ps_s[:, base + W1 - P : base + W1],
                            ps_s[:, base + W1 - P : base + W1],
                            mask_sb[:, j, ri * P : (ri + 1) * P],
                        )
                        mp = misc_p.tile([P, 1], F32, tag="negmax", bufs=8)
                        nc.vector.reduce_max(mp[:], ps_s[:, base : base + W1],
                                             axis=mybir.AxisListType.X,
                                             negate=True)
                        mparts.append(mp)
                    negmax = mparts[0]
                    nc.vector.tensor_tensor(negmax[:], negmax[:], mparts[1][:],
                                            op=mybir.AluOpType.min)
                    # softmax over both ranges (3D AP [P, 2, W1]); the attn
                    # tile is sized to 2*W1 columns (small slots get a small
                    # tag) so 8 tiles fit in SBUF at once
                    sc2 = ps_s[:].rearrange("p (r s) -> p r s", s=SO)[:, :, :W1]
                    if W1 <= KH:
                        attn = attn_p.tile([P, 2, KH], BF16, tag="attnS", bufs=4)
                    else:
                        attn = attn_p.tile([P, 2, SO], BF16, tag="attnB", bufs=4)
                    lsum = misc_p.tile([P, 1], F32, tag="lsum", bufs=8)
                    nc.scalar.activation(attn[:, :, :W1], sc2,
                                         mybir.ActivationFunctionType.Exp,
                                         bias=negmax[:], scale=1.0,
                                         accum_out=lsum[:])
                    inv_l = misc_p.tile([P, 1], F32, tag="invl", bufs=8)
                    nc.vector.reciprocal(inv_l[:], lsum[:])
                    pend[j] = (attn, inv_l)

                def attn_tail(j):
                    attn, inv_l = pend.pop(j)
                    attnT = attnT_p.tile([P, 2 * NJ, P], BF16, tag="attnT")
                    for r in range(2):
                        for kk in range(j + 1):
                            pe_transpose(
                                attnT[:, r * NJ + kk, :],
                                attn[:, r, kk * P : (kk + 1) * P],
                            )
                    # attn @ v -> h_attn [q, D] (natural), deferred 1/l scale
                    ps_av = ps_av_p.tile([P, 1024], F32, tag="av")
                    for off, n in _chunks(D, 512):
                        first = True
                        for r in range(2):
                            for kk in range(j + 1):
                                g = r * NJ + kk
                                nc.tensor.matmul(
                                    ps_av[:, off : off + n],
                                    attnT[:, g, :],
                                    v_q[g // MH][:, g % MH, off : off + n],
                                    start=first,
                                    stop=(r == 1 and kk == j),
                                )
                                first = False
                    h_attn = misc_p.tile([P, D], BF16, tag="hattn")
                    nc.vector.tensor_scalar_mul(h_attn[:], ps_av[:, :D], inv_l[:])
                    # transpose into h_attnT columns for this slot
                    for i in range(ND):
                        pe_transpose(
                            h_attnT[:, i, j * P : (j + 1) * P],
                            h_attn[:, i * P : (i + 1) * P],
                        )

                # all heads, then all tails: k (both halves) is gathered
                # before any v half lands, so every head is runnable before
                # the first tail's v dependency — the PE FIFO never parks a
                # v-dependent tail in front of runnable heads
                for j in range(NJ):
                    attn_head(j)
                for j in range(NJ):
                    attn_tail(j)

                if stage == "attn":
                    nc.sync.dma_start(dbg[:, :, :SO], h_attnT[:])
                    break

                # ---------------- Wo + residual -> h_resT
                h_resT = res_p.tile([P, ND, SO], BF16, tag="res")
                for i in range(ND):
                    for off, n in _chunks(SO, 512):
                        ps = ps_mm.tile([P, 512], F32, tag="mm")
                        for k in range(ND):
                            nc.tensor.matmul(
                                ps[:, :n],
                                wo_sb[:, k, i * P : (i + 1) * P],
                                h_attnT[:, k, off : off + n],
                                start=(k == 0),
                                stop=(k == ND - 1),
                            )
                        nc.vector.tensor_add(
                            h_resT[:, i, off : off + n], ps[:, :n],
                            own_hT[:, i, off : off + n],
                        )

                # ---------------- FFN (per q-half; streamed w1/w2)
                # block 2 runs halves in reverse so the half holding the
                # final token finishes first -> the logits AllGather and
                # w_out streaming overlap the remaining FFN work
                own_hT_next = own_p.tile([P, ND, SO], BF16, tag="own")
                if l == 2 and stage == "full":
                    last_col = misc_p.tile([P, ND], BF16, tag="lastcol")
                n_w1ch = (NH + W1CH - 1) // W1CH
                qchunks = _chunks(SO, QH)
                if l == 2:
                    qchunks = qchunks[::-1]
                NHQ = NH // 4           # h-blocks per midT quarter
                for qoff, qn in qchunks:
                    midT = [big.tile([P, NHQ, QH], BF16, tag=f"kv{q}", name=f"midT{q}")
                            for q in range(4)]
                    for ch in range(n_w1ch):
                        hb0 = ch * W1CH
                        nhb = min(W1CH, NH - hb0)
                        w1_sb = w_p.tile([P, ND, W1CH * P], BF16, tag="w")
                        nc.scalar.dma_start(w1_sb[:, :, : nhb * P], wts[l, "w1"][ch])
                        for hb in range(nhb):
                            g = hb0 + hb
                            ps = ps_mm.tile([P, 512], F32, tag="mm")
                            for k in range(ND):
                                nc.tensor.matmul(
                                    ps[:, :qn],
                                    w1_sb[:, k, hb * P : (hb + 1) * P],
                                    h_resT[:, k, qoff : qoff + qn],
                                    start=(k == 0),
                                    stop=(k == ND - 1),
                                )
                            nc.vector.tensor_scalar_max(
                                midT[g // NHQ][:, g % NHQ, :qn], ps[:, :qn], 0.0,
                            )
                    for i in range(ND):
                        w2_sb = w_p.tile([P, NH, P], BF16, tag="w")
                        nc.sync.dma_start(w2_sb[:], wts[l, "w2"][i])
                        ps = ps_mm.tile([P, 512], F32, tag="mm")
                        for hb in range(NH):
                            nc.tensor.matmul(
                                ps[:, :qn],
                                w2_sb[:, hb, :],
                                midT[hb // NHQ][:, hb % NHQ, :qn],
                                start=(hb == 0),
                                stop=(hb == NH - 1),
                            )
                        nc.vector.tensor_add(
                            own_hT_next[:, i, qoff : qoff + qn], ps[:, :qn],
                            h_resT[:, i, qoff : qoff + qn],
                        )
                        if l == 2 and stage == "full" and qoff + qn == SO:
                            # last token's activations, kept in a tiny tile so
                            # the logits path doesn't wait on the whole FFN
                            nc.vector.tensor_add(
                                last_col[:, i : i + 1], ps[:, qn - 1 : qn],
                                h_resT[:, i, SO - 1 : SO],
                            )
                    if l == 2 and stage == "full" and qoff + qn == SO:
                        # emit the logits AllGather prologue here so the PE
                        # transposes interleave into the remaining FFN work
                        # and the 8-core collective overlaps it
                        lgp = logits_prologue()
                own_hT = own_hT_next
                if stage == "block1":
                    nc.sync.dma_start(dbg[:, :, :SO], own_hT[:])
                    break

            if stage == "blocks":
                nc.sync.dma_start(dbg[:, :, :SO], own_hT[:])

            if stage == "full":
                lhsT = logits_lhsT(lgp)
                # w_out streamed in VC-wide chunks through the kv pool slots;
                # the first chunks' slots free mid-way through block-2 FFN so
                # most of the stream overlaps compute
                wo_ts = []
                for ci, (off, n) in enumerate(_chunks(VS, VC)):
                    wo_t = big.tile([P, ND, VC], BF16, tag=f"kv{ci % 4}", name=f"wot{ci}")
                    eng = nc.sync if ci % 2 == 0 else nc.scalar
                    eng.dma_start(wo_t[:, :, :n], w_out[ci])
                    wo_ts.append(wo_t)
                # col-tiled: 4 vocab chunks run concurrently in 4 distinct
                # 32-wide column groups of the PE array (M=4 each), each
                # accumulating in its own bank of the (now idle) sc PSUM tile
                lchunks = _chunks(VS, VC)
                for g0 in range(0, len(lchunks), 4):
                    grp = lchunks[g0 : g0 + 4]
                    psg = ps_sc.tile([P, S], F32, tag="sc", name=f"lgps{g0}")
                    for k in range(ND):
                        for t, (off, n) in enumerate(grp):
                            nc.tensor.matmul(
                                psg[32 * t : 32 * t + 4, 512 * t : 512 * t + n],
                                lhsT[:, k, :], wo_ts[g0 + t][:, k, :n],
                                start=(k == 0), stop=(k == ND - 1),
                                tile_position=(0, 32 * t),
                            )
                    for t, (off, n) in enumerate(grp):
                        lg = misc_p.tile([4, VC], F32, tag="lg", bufs=2)
                        nc.vector.tensor_copy(
                            lg[:, :n],
                            psg[32 * t : 32 * t + 4, 512 * t : 512 * t + n],
                        )
                        nc.sync.dma_start(logits[:, off : off + n], lg[:, :n])

    nc.compile()
    return nc


# ----------------------------------------------------------------------------
# host side
# ----------------------------------------------------------------------------

def make_in_maps(tokens, emb, pe, weights, S=2048, D=1024, H=4096, V=32000,
                 n_cores=8):
    """weights: dict with l{1,2}_{wk,wv,wo,w1,w2} and w_out (fp32 numpy)."""
    bf = ml_dtypes.bfloat16
    NJ = (S // P) // 2
    VS = V // n_cores
    ND, NH = D // P, H // P
    CW = min(8, NH) * P
    NCH = H // CW
    VC = 500 if VS % 500 == 0 else VS
    NC_OUT = VS // VC
    emb_f = np.ascontiguousarray(emb, dtype=np.float32)
    pe_f = np.asarray(pe, dtype=np.float32)
    scale = 1.0 / np.sqrt(float(D))

    def pmaj(w):
        # [K, N] -> [P, K//P, N]: per-partition-contiguous weight layout
        K, N = w.shape
        return np.ascontiguousarray(w.reshape(K // P, P, N).transpose(1, 0, 2))

    w_bf = {}
    for l in (1, 2):
        wk = np.asarray(weights[f"l{l}_wk"], np.float32) * scale
        w_bf[f"l{l}_wk"] = pmaj(wk).astype(bf)
        w_bf[f"l{l}_wv"] = pmaj(np.asarray(weights[f"l{l}_wv"], np.float32)).astype(bf)
        w_bf[f"l{l}_wo"] = pmaj(np.asarray(weights[f"l{l}_wo"], np.float32)).astype(bf)
        w1 = np.asarray(weights[f"l{l}_w1"], np.float32)      # [D, H]
        w_bf[f"l{l}_w1"] = np.ascontiguousarray(
            w1.reshape(ND, P, NCH, CW).transpose(2, 1, 0, 3)
        ).astype(bf)                                          # [NCH, P, ND, CW]
        w2 = np.asarray(weights[f"l{l}_w2"], np.float32)      # [H, D]
        w_bf[f"l{l}_w2"] = np.ascontiguousarray(
            w2.reshape(NH, P, ND, P).transpose(2, 1, 0, 3)
        ).astype(bf)                                          # [ND, P, NH, P]
    w_out_f = np.asarray(weights["w_out"], np.float32)        # [D, V]

    tokens = np.asarray(tokens)
    B = tokens.shape[0]
    in_maps = []
    tri = np.triu(np.full((P, P), NEG, np.float32), k=1)  # [q, k] mask
    for c in range(n_cores):
        b, t = c // 2, c % 2
        own_rows = np.concatenate(
            [np.arange((2 * j + t) * P, (2 * j + t + 1) * P) for j in range(NJ)]
        )
        tok_own = tokens[b, own_rows].astype(np.int64)
        h0 = emb_f[tok_own] + pe_f[own_rows]                  # [SO, D]
        ND = D // P
        h0T_own = np.ascontiguousarray(
            h0.T.reshape(ND, P, len(own_rows)).transpose(1, 0, 2)
        ).astype(bf)                                          # [P, ND, SO]
        mask = np.zeros((NJ, P, 2 * P), np.float32)
        for j in range(NJ):
            if t == 0:
                mask[j, :, :P] = tri
                mask[j, :, P:] = NEG
            else:
                mask[j, :, P:] = tri
        w_out_c = w_out_f[:, c * VS : (c + 1) * VS]           # [D, VS]
        w_out_c = np.ascontiguousarray(
            w_out_c.reshape(ND, P, NC_OUT, VC).transpose(2, 1, 0, 3)
        ).astype(bf)                                          # [NC_OUT, P, ND, VC]
        in_map = {
            "h0T_own": h0T_own,
            "mask": np.ascontiguousarray(mask.transpose(1, 0, 2)).astype(bf),
            "w_out": w_out_c,
        }
        in_map.update(w_bf)
        in_maps.append(in_map)
    return in_maps


_NC_CACHE = {}


def _get_nc(key=(2048, 1024, 4096, 32000, 8)):
    if key not in _NC_CACHE:
        _NC_CACHE[key] = build_nc(*key)
    return _NC_CACHE[key]


def kernel(tokens, emb, pe, l1_wk, l1_wv, l1_wo, l1_w1, l1_w2,
           l2_wk, l2_wv, l2_wo, l2_w1, l2_w2, w_out):
    S = int(np.asarray(tokens).shape[1])
    D = int(np.asarray(emb).shape[1])
    H = int(np.asarray(l1_w1).shape[1])
    V = int(np.asarray(emb).shape[0])
    n_cores = 8
    nc = _get_nc((S, D, H, V, n_cores))
    weights = dict(
        l1_wk=l1_wk, l1_wv=l1_wv, l1_wo=l1_wo, l1_w1=l1_w1, l1_w2=l1_w2,
        l2_wk=l2_wk, l2_wv=l2_wv, l2_wo=l2_wo, l2_w1=l2_w1, l2_w2=l2_w2,
        w_out=w_out,
    )
    in_maps = make_in_maps(tokens, emb, pe, weights, S, D, H, V, n_cores)
    try:
        res = run_bass_kernel_spmd(nc, in_maps, core_ids=list(range(n_cores)))
    except Exception:
        # a previous crashed run can leave the device wedged; one retry
        # (fresh NRT session) clears it
        import os
        os.environ.setdefault("NEURON_RT_RESET_CORES", "1")
        res = run_bass_kernel_spmd(nc, in_maps, core_ids=list(range(n_cores)))
    VS = V // n_cores
    out = np.zeros((np.asarray(tokens).shape[0], V), np.float32)
    for c in range(n_cores):
        out[:, c * VS : (c + 1) * VS] = res.results[c]["logits"]
    return out

